# revision 1
# baseline (speedup 1.0000x reference)
"""Trainium2 Bass kernel for multi-head cross-attention block (nn_MCA).

Math (per batch b):
  q  = Wq  @ xq[b]   (1x1 conv)      k,v = Wkv @ x[b]
  per head h (32 heads, dh=8): attn = softmax(q_h^T k_h / sqrt(8))
  out = Wproj @ concat_h(attn @ v_h) + bias

Sharding: 8 cores = (batch b in 0..4) x (head-half in 0..2); each core handles
16 heads of one batch and produces a partial [256,1024] projection output;
host sums the two halves per batch and adds bias.

Device layout:
  - scores^T computed as [k_tok, q_tok] psum tiles with K=dh=8 contraction;
    4 heads run CONCURRENTLY in the PE array via 32-row tile_position groups
    (heads live at 32-aligned partition offsets of scattered qT/kT tiles:
    partition 32g+d of tile j <-> local head 4j+g, dim d).
  - exp on ScalarE reads 4 psum banks [128,2048] at once (amortizes ACT
    instruction overhead); the 1/sqrt(8) scale is folded into the ACT affine.
    ScalarE is the bottleneck engine (~16.8M exp elements per core); the
    whole schedule exists to keep it 100% busy.
  - attn@v computed transposed with a ones-augmented V (M=9 stationary),
    giving the softmax denominator for free; 4 heads packed via 32-col
    tile_position into one psum bank.
  - all psum usage shares one 2-slot x 4-bank pool so j=1..3 q/k/v
    projections can be deferred into the first exp stream (short startup).
  - normalization (1/sum) applied once at the end on [128,1024] via a
    partition-broadcast DMA + one multiply; projection partial stays on-core.
"""
import numpy as np

B, C = 4, 256
HEADS, DH = 32, 8
N = 1024                    # tokens (32*32), both for q and kv
SCALE = DH ** -0.5
NCORES = 8
NKT = 8                     # k tiles of 128 tokens
NQH = 2                     # q halves of 512 tokens
NJ = 4                      # rounds of 4 heads

_cache = {}


def _build():
    if "nc" in _cache:
        return _cache["nc"]
    import concourse.mybir as mybir
    import concourse.tile as tile
    from concourse import bacc

    F32 = mybir.dt.float32
    EXP = mybir.ActivationFunctionType.Exp

    nc = bacc.Bacc("TRN2", target_bir_lowering=False, debug=False,
                   num_devices=NCORES)
    mm = nc.tensor.matmul

    xq_d = nc.dram_tensor("xq", [C, N], F32, kind="ExternalInput")
    x_d = nc.dram_tensor("x", [C, N], F32, kind="ExternalInput")
    wq_d = nc.dram_tensor("wq", [C, 512], F32, kind="ExternalInput")   # scattered cols
    wk_d = nc.dram_tensor("wk", [C, 512], F32, kind="ExternalInput")   # scattered cols
    wv_d = nc.dram_tensor("wv", [C, 128], F32, kind="ExternalInput")   # plain cols
    wp_d = nc.dram_tensor("wp", [128, C], F32, kind="ExternalInput")
    out_d = nc.dram_tensor("out", [C, N], F32, kind="ExternalOutput")
    dbg = {}
    if _cache.get("debug"):
        for nm, shp in [("qT_o", [128, 4096]), ("kT_o", [128, 4096]),
                        ("v9_o", [128, NKT * 144]), ("cat_o", [128, N]),
                        ("s_o", [16, N]), ("e_o", [128, 2048]),
                        ("rb_o", [128, N])]:
            dbg[nm] = nc.dram_tensor(nm, shp, F32, kind="ExternalOutput")

    REP = _cache.get("repeat", 1)
    interleave = REP == 1

    with tile.TileContext(nc) as tc:
        from contextlib import ExitStack
        with ExitStack() as st:
            pp = st.enter_context(tc.tile_pool(name="persist", bufs=1))
            xq_sb = pp.tile([128, 2048], F32, name="xq_sb")   # chunk c at c*1024
            x_sb = pp.tile([128, 2048], F32, name="x_sb")
            wq_sb = pp.tile([128, 1024], F32, name="wq_sb")   # chunk c at c*512
            wk_sb = pp.tile([128, 1024], F32, name="wk_sb")
            wv_sb = pp.tile([128, 256], F32, name="wv_sb")    # chunk c at c*128
            wp_sb = pp.tile([128, 256], F32, name="wp_sb")
            qT = pp.tile([128, 4096], F32, name="qT")         # tile j at j*1024
            kT = pp.tile([128, 4096], F32, name="kT")
            v9 = pp.tile([128, NKT * 144], F32, name="v9")    # [ktok, kt*144 + h*9 + d]
            attn_cat = pp.tile([128, N], F32, name="attn_cat")
            s_cat = pp.tile([16, N], F32, name="s_cat")
            r_cat = pp.tile([16, N], F32, name="r_cat")
            rb = pp.tile([128, N], F32, name="rb")
            attn_n = pp.tile([128, N], F32, name="attn_n")

            # --- input DMAs: what the j=0 projections need goes first ---
            for c in range(2):
                nc.sync.dma_start(out=xq_sb[:, c * 1024:(c + 1) * 1024],
                                  in_=xq_d.ap()[c * 128:(c + 1) * 128, :])
                nc.sync.dma_start(out=x_sb[:, c * 1024:(c + 1) * 1024],
                                  in_=x_d.ap()[c * 128:(c + 1) * 128, :])
                nc.sync.dma_start(out=wq_sb[:, c * 512:c * 512 + 128],
                                  in_=wq_d.ap()[c * 128:(c + 1) * 128, 0:128])
                nc.sync.dma_start(out=wk_sb[:, c * 512:c * 512 + 128],
                                  in_=wk_d.ap()[c * 128:(c + 1) * 128, 0:128])
            for c in range(2):
                nc.sync.dma_start(out=wq_sb[:, c * 512 + 128:(c + 1) * 512],
                                  in_=wq_d.ap()[c * 128:(c + 1) * 128, 128:512])
                nc.sync.dma_start(out=wk_sb[:, c * 512 + 128:(c + 1) * 512],
                                  in_=wk_d.ap()[c * 128:(c + 1) * 128, 128:512])
                nc.sync.dma_start(out=wv_sb[:, c * 128:(c + 1) * 128],
                                  in_=wv_d.ap()[c * 128:(c + 1) * 128, :])
            nc.sync.dma_start(out=wp_sb, in_=wp_d.ap())
            nc.vector.memset(v9, 1.0)

            # one shared psum pool: 2 slots x 4 banks
            sp = st.enter_context(tc.tile_pool(name="smm", bufs=2, space="PSUM"))
            ep = st.enter_context(
                tc.tile_pool(name="epool", bufs=_cache.get("ebufs", 10)))

            def proj_qk(j):
                for name, w_sb, src, dst in (("q", wq_sb, xq_sb, qT),
                                             ("k", wk_sb, x_sb, kT)):
                    for qh in range(NQH):
                        ps = sp.tile([128, 512], F32,
                                     name=f"ps{name}{j}{qh}", tag="s")
                        for cc in range(2):
                            mm(out=ps,
                               lhsT=w_sb[:, cc * 512 + 128 * j:
                                         cc * 512 + 128 * j + 128],
                               rhs=src[:, cc * 1024 + qh * 512:
                                       cc * 1024 + (qh + 1) * 512],
                               start=(cc == 0), stop=(cc == 1))
                        nc.vector.tensor_copy(
                            dst[:, j * 1024 + qh * 512:
                                j * 1024 + (qh + 1) * 512], ps)

            def proj_v():
                for kt in range(NKT):
                    ps = sp.tile([128, 128], F32, name=f"psv{kt}", tag="s")
                    for cc in range(2):
                        mm(out=ps,
                           lhsT=x_sb[:, cc * 1024 + kt * 128:
                                     cc * 1024 + (kt + 1) * 128],
                           rhs=wv_sb[:, cc * 128:(cc + 1) * 128],
                           start=(cc == 0), stop=(cc == 1))
                    nc.vector.tensor_copy(
                        v9[:, kt * 144:(kt + 1) * 144].rearrange(
                            "p (h d) -> p h d", d=9)[:, :, 0:8],
                        ps.rearrange("p (h d) -> p h d", d=8))

            def scores_exp(rep, qh, j):
                e_tiles = []
                for kt in range(NKT):
                    ps_s = sp.tile([128, 2048], F32,
                                   name=f"s{rep}_{qh}{j}{kt}", tag="s")
                    for g in range(4):
                        mm(out=ps_s[:, g * 512:(g + 1) * 512],
                           lhsT=kT[32 * g:32 * g + 8,
                                   j * 1024 + kt * 128:
                                   j * 1024 + (kt + 1) * 128],
                           rhs=qT[32 * g:32 * g + 8,
                                  j * 1024 + qh * 512:
                                  j * 1024 + (qh + 1) * 512],
                           start=True, stop=True,
                           tile_position=(32 * g, 0))
                    e = ep.tile([128, 2048], F32,
                                name=f"e{rep}_{qh}{j}{kt}", tag="e")
                    nc.scalar.activation(out=e, in_=ps_s, func=EXP, scale=SCALE)
                    if dbg and rep == 0 and qh == 0 and j == 0 and kt == 0:
                        nc.sync.dma_start(out=dbg["e_o"].ap(), in_=e)
                    e_tiles.append(e)
                return e_tiles

            def attnv(rep, qh, j, e_tiles):
                ps_o = sp.tile([128, 512], F32, name=f"o{rep}_{qh}{j}", tag="s")
                for kt in range(NKT):
                    for g in range(4):
                        mm(out=ps_o[32 * g:32 * g + 9, :],
                           lhsT=v9[:, kt * 144 + (4 * j + g) * 9:
                                   kt * 144 + (4 * j + g) * 9 + 9],
                           rhs=e_tiles[kt][:, g * 512:(g + 1) * 512],
                           start=(kt == 0), stop=(kt == NKT - 1),
                           tile_position=(0, 32 * g))
                o_st = ep.tile([128, 512], F32, name=f"ost{rep}_{qh}{j}",
                               tag="ost")
                nc.vector.tensor_copy(o_st, ps_o)
                # only AP dim 0 crosses partitions -> one DMA per 32-row group
                for g in range(4):
                    nc.sync.dma_start(
                        out=attn_cat[32 * j + 8 * g:32 * j + 8 * g + 8,
                                     qh * 512:(qh + 1) * 512],
                        in_=o_st[32 * g:32 * g + 8, :])
                    nc.sync.dma_start(
                        out=s_cat[4 * j + g:4 * j + g + 1,
                                  qh * 512:(qh + 1) * 512],
                        in_=o_st[32 * g + 8:32 * g + 9, :])

            if interleave:
                # j=0 projections, then round (0,0) scores immediately; defer
                # the remaining projections into the first exp stream.
                proj_qk(0)
                e00 = scores_exp(0, 0, 0)
                for j in range(1, NJ):
                    proj_qk(j)
                proj_v()
                attnv(0, 0, 0, e00)
                rounds = [(qh, j) for qh in range(NQH) for j in range(NJ)][1:]
                for qh, j in rounds:
                    attnv(0, qh, j, scores_exp(0, qh, j))
            else:
                for j in range(NJ):
                    proj_qk(j)
                proj_v()
                with tc.For_i(0, REP):
                    for qh in range(NQH):
                        for j in range(NJ):
                            attnv(0, qh, j, scores_exp(0, qh, j))

            if dbg:
                nc.sync.dma_start(out=dbg["qT_o"].ap(), in_=qT)
                nc.sync.dma_start(out=dbg["kT_o"].ap(), in_=kT)
                nc.sync.dma_start(out=dbg["v9_o"].ap(), in_=v9)
                nc.sync.dma_start(out=dbg["cat_o"].ap(), in_=attn_cat)
                nc.sync.dma_start(out=dbg["s_o"].ap(), in_=s_cat)

            # ---- tail: normalize + projection ----
            nc.vector.reciprocal(r_cat, s_cat)
            nc.gpsimd.dma_start(out=rb,
                                in_=r_cat.unsqueeze(1).broadcast_to([16, 8, N]))
            if dbg:
                nc.sync.dma_start(out=dbg["rb_o"].ap(), in_=rb)
            nc.vector.tensor_mul(attn_n, attn_cat, rb)
            out_sb = pp.tile([128, 2048], F32, name="out_sb")
            for ot in range(2):
                for qh in range(NQH):
                    ps_p = sp.tile([128, 512], F32, name=f"pp{ot}{qh}", tag="s")
                    mm(out=ps_p,
                       lhsT=wp_sb[:, ot * 128:(ot + 1) * 128],
                       rhs=attn_n[:, qh * 512:(qh + 1) * 512],
                       start=True, stop=True)
                    nc.vector.tensor_copy(
                        out_sb[:, ot * 1024 + qh * 512:
                               ot * 1024 + (qh + 1) * 512], ps_p)
            for ot in range(2):
                nc.sync.dma_start(
                    out=out_d.ap()[ot * 128:(ot + 1) * 128, :],
                    in_=out_sb[:, ot * 1024:(ot + 1) * 1024])

    nc.compile()
    _cache["nc"] = nc
    return nc


def _prep_core(core, xq, x, Wq, Wkv, Wproj):
    half = core % 2
    b = core // 2
    xq_np = np.ascontiguousarray(xq[b].reshape(C, N))
    x_np = np.ascontiguousarray(x[b].reshape(C, N))

    # scattered column permutation: local head h=4j+g, dim d -> col 128j+32g+d
    hl = np.arange(16)
    d = np.arange(8)
    colperm = (128 * (hl[:, None] // 4) + 32 * (hl[:, None] % 4)
               + d[None, :]).reshape(-1)

    wq_block = Wq[128 * half:128 * half + 128, :]          # rows 8h+d
    wq_scat = np.zeros((C, 512), np.float32)
    wq_scat[:, colperm] = wq_block.T
    wk_block = Wkv[128 * half:128 * half + 128, :]
    wk_scat = np.zeros((C, 512), np.float32)
    wk_scat[:, colperm] = wk_block.T
    wv_rhs = np.ascontiguousarray(
        Wkv[256 + 128 * half:256 + 128 * half + 128, :].T)
    wp = np.ascontiguousarray(Wproj[:, 128 * half:128 * half + 128].T)
    return {"xq": xq_np, "x": x_np, "wq": wq_scat, "wk": wk_scat,
            "wv": wv_rhs, "wp": wp}


def run_internal(inputs, trace=False):
    from concourse.bass_utils import run_bass_kernel_spmd
    nc = _build()
    xq, x = np.asarray(inputs["xq"]), np.asarray(inputs["x"])
    Wq, Wkv = np.asarray(inputs["Wq"]), np.asarray(inputs["Wkv"])
    Wproj, bproj = np.asarray(inputs["Wproj"]), np.asarray(inputs["bproj"])
    in_maps = [_prep_core(c, xq, x, Wq, Wkv, Wproj) for c in range(NCORES)]
    res = run_bass_kernel_spmd(nc, in_maps, list(range(NCORES)), trace=trace)
    out = np.zeros((B, C, 32, 32), np.float32)
    for b in range(B):
        part = res.results[2 * b]["out"] + res.results[2 * b + 1]["out"]
        out[b] = (part + bproj[:, None]).reshape(C, 32, 32)
    return out, res


def kernel(**inputs):
    out, _ = run_internal(inputs, trace=False)
    return out



# revision 15
# speedup vs baseline: 49.5149x; 49.5149x over previous
"""Trainium2 Bass kernel for multi-head cross-attention block (nn_MCA).

Math (per batch b):
  q  = Wq  @ xq[b]   (1x1 conv)      k,v = Wkv @ x[b]
  per head h (32 heads, dh=8): attn = softmax(q_h^T k_h / sqrt(8))
  out = Wproj @ concat_h(attn @ v_h) + bias

Sharding: 8 cores = (batch b in 0..4) x (q-token half t in 0..2). Each core
handles ALL 32 heads for 512 query tokens of one batch, so per-core outputs
are disjoint [256, 512] slices -- no partial sums on the host.

Wall-clock time here is dominated by host<->device transfer through the
axon tunnel (~35 MB/s, ~70ms/dispatch), so the kernel is built to move as
few bytes as possible:
  - all tensors cross the wire as bf16 (rel err ~5e-3, well under 2e-2);
  - xq is sharded disjointly; x duplicated only x2 (both halves of a batch
    need all kv tokens); weights shipped DENSE (the column-scatter needed
    for head-packed matmuls is done on-device with strided vector copies);
  - donated output buffers are created on-device (never transferred);
  - the jitted executable is AOT-compiled once and cached in module state.

Device schedule (same structure as the tuned 16-head baseline, now 32 heads
x 512 q-tokens -- identical engine volumes):
  - scores^T as [k_tok, q_tok] psum tiles with K=dh=8 contraction; 4 heads
    run concurrently in the PE array via 32-row tile_position groups.
  - exp on ScalarE reads 4 psum banks [128,2048] at once; the 1/sqrt(8)
    scale is folded into the ACT affine. ScalarE (16.8M exp elems) is the
    bottleneck engine.
  - attn@v transposed with a ones-augmented V (M=9 stationary) giving the
    softmax denominator for free; 4 heads packed via 32-col tile_position.
  - normalization (1/sum) applied once at the end via a partition-broadcast
    DMA + one multiply; projection output leaves as bf16 [256, 512].
"""
import numpy as np

B, C = 4, 256
HEADS, DH = 32, 8
N = 1024                    # kv tokens (32*32)
NQ = 512                    # q tokens per core (half of 1024)
SCALE = DH ** -0.5
NCORES = 8
NKT = 8                     # k tiles of 128 tokens
NJ = 8                      # rounds of 4 heads (32 heads total)

_cache = {}


def _build():
    if "nc" in _cache:
        return _cache["nc"]
    import concourse.mybir as mybir
    import concourse.tile as tile
    from concourse import bacc

    F32 = mybir.dt.float32
    BF16 = mybir.dt.bfloat16
    EXP = mybir.ActivationFunctionType.Exp

    nc = bacc.Bacc("TRN2", target_bir_lowering=False, debug=False,
                   num_devices=NCORES)
    mm = nc.tensor.matmul

    # Per-core inputs are 1/8 shards; full tensors are assembled on-device
    # via AllGather (weights: 8-way; k/v: within the batch pair).
    xq_d = nc.dram_tensor("xq", [C, NQ], BF16, kind="ExternalInput")
    xh_d = nc.dram_tensor("x", [C, NQ], BF16, kind="ExternalInput")
    w_d = nc.dram_tensor("w", [32, 1024], BF16, kind="ExternalInput")
    out_d = nc.dram_tensor("out", [C, NQ], BF16, kind="ExternalOutput")

    with tile.TileContext(nc) as tc:
        from contextlib import ExitStack
        with ExitStack() as st:
            pp = st.enter_context(tc.tile_pool(name="persist", bufs=1))
            xq_sb = pp.tile([128, 1024], BF16, name="xq_sb")  # chunk c @ c*512
            xh_sb = pp.tile([128, 1024], BF16, name="xh_sb")  # chunk c @ c*512
            wqd = pp.tile([128, 512], BF16, name="wqd")       # chunk c @ c*256
            wkd = pp.tile([128, 512], BF16, name="wkd")
            wvd = pp.tile([128, 512], BF16, name="wvd")
            wpd = pp.tile([128, 512], BF16, name="wpd")
            wqs = pp.tile([128, 2048], BF16, name="wqs")      # scattered, c @ c*1024
            wks = pp.tile([128, 2048], BF16, name="wks")
            qT = pp.tile([128, NJ * NQ], BF16, name="qT")     # tile j @ j*512
            kh = pp.tile([128, NJ * 512], BF16, name="kh")    # my k half, j @ j*512
            v9h = pp.tile([128, 4 * 288], BF16, name="v9h")   # my v half, 4 kt tiles
            kT = pp.tile([128, NJ * N], BF16, name="kT")      # tile j @ j*1024
            v9 = pp.tile([128, NKT * 288], BF16, name="v9")   # [ktok, kt*288+h*9+d]
            attn_cat = pp.tile([128, 1024], F32, name="attn_cat")  # chunk cc @ cc*512
            s_cat = pp.tile([32, NQ], F32, name="s_cat")
            r_cat = pp.tile([32, NQ], F32, name="r_cat")
            rb = pp.tile([128, 1024], F32, name="rb")
            attn_n = pp.tile([128, 1024], BF16, name="attn_n")
            out_sb = pp.tile([128, 1024], BF16, name="out_sb")

            dp = st.enter_context(tc.tile_pool(name="dram", bufs=1,
                                               space="DRAM"))
            wb_in = dp.tile([32, 1024], BF16, name="wb_in")
            wb_out = dp.tile([C, 1024], BF16, name="wb_out",
                             addr_space="Shared")
            kvb_in = dp.tile([128, 5248], BF16, name="kvb_in")
            kvb_out = dp.tile([256, 5248], BF16, name="kvb_out")

            # --- weight AllGather first: everything depends on it ---
            nc.gpsimd.dma_start(out=wb_in[:], in_=w_d.ap())
            nc.gpsimd.collective_compute(
                "AllGather", mybir.AluOpType.bypass,
                replica_groups=[list(range(NCORES))],
                ins=[wb_in.opt()], outs=[wb_out.opt()])
            for c in range(2):
                for i, wt in enumerate((wqd, wkd, wvd, wpd)):
                    nc.sync.dma_start(
                        out=wt[:, c * 256:(c + 1) * 256],
                        in_=wb_out[c * 128:(c + 1) * 128,
                                   i * 256:(i + 1) * 256])
            for c in range(2):
                nc.sync.dma_start(out=xq_sb[:, c * 512:(c + 1) * 512],
                                  in_=xq_d.ap()[c * 128:(c + 1) * 128, :])
                nc.sync.dma_start(out=xh_sb[:, c * 512:(c + 1) * 512],
                                  in_=xh_d.ap()[c * 128:(c + 1) * 128, :])
            nc.vector.memset(v9h, 1.0)

            # on-device column scatter: dense col 32j+8g+d -> 128j+32g+d
            # (cols 8..31 of each 32-group stay junk; matmuls never read them)
            for w_s, w_dn in ((wqs, wqd), (wks, wkd)):
                for c in range(2):
                    dst = w_s[:, c * 1024:(c + 1) * 1024].rearrange(
                        "p (jg dd) -> p jg dd", dd=32)[:, :, 0:8]
                    src = w_dn[:, c * 256:(c + 1) * 256].rearrange(
                        "p (jg d) -> p jg d", d=8)
                    nc.vector.tensor_copy(dst, src)

            sp = st.enter_context(tc.tile_pool(name="smm", bufs=2, space="PSUM"))
            ep = st.enter_context(tc.tile_pool(name="epool", bufs=10))

            def proj_q(j):
                ps = sp.tile([128, 512], F32, name=f"psq{j}", tag="s")
                for c in range(2):
                    mm(out=ps,
                       lhsT=wqs[:, c * 1024 + 128 * j:c * 1024 + 128 * j + 128],
                       rhs=xq_sb[:, c * 512:(c + 1) * 512],
                       start=(c == 0), stop=(c == 1))
                nc.vector.tensor_copy(qT[:, j * NQ:(j + 1) * NQ], ps)

            def proj_k(j):
                ps = sp.tile([128, 512], F32, name=f"psk{j}", tag="s")
                for c in range(2):
                    mm(out=ps,
                       lhsT=wks[:, c * 1024 + 128 * j:
                                c * 1024 + 128 * j + 128],
                       rhs=xh_sb[:, c * 512:(c + 1) * 512],
                       start=(c == 0), stop=(c == 1))
                nc.vector.tensor_copy(kh[:, j * 512:(j + 1) * 512], ps)

            def proj_v():
                for kt in range(4):
                    ps = sp.tile([128, 256], F32, name=f"psv{kt}", tag="s")
                    for c in range(2):
                        mm(out=ps,
                           lhsT=xh_sb[:, c * 512 + kt * 128:
                                    c * 512 + (kt + 1) * 128],
                           rhs=wvd[:, c * 256:(c + 1) * 256],
                           start=(c == 0), stop=(c == 1))
                    nc.vector.tensor_copy(
                        v9h[:, kt * 288:(kt + 1) * 288].rearrange(
                            "p (h dd) -> p h dd", dd=9)[:, :, 0:8],
                        ps.rearrange("p (h d) -> p h d", d=8))

            def gather_kv():
                # pair-AllGather my [kh | v9h] -> full-token kT / v9
                nc.sync.dma_start(out=kvb_in[:, 0:4096], in_=kh)
                nc.sync.dma_start(out=kvb_in[:, 4096:5248], in_=v9h)
                nc.gpsimd.collective_compute(
                    "AllGather", mybir.AluOpType.bypass,
                    replica_groups=[[2 * b, 2 * b + 1] for b in range(B)],
                    ins=[kvb_in.opt()], outs=[kvb_out.opt()])
                for j in range(NJ):
                    for h in range(2):
                        nc.sync.dma_start(
                            out=kT[:, j * N + h * 512:j * N + (h + 1) * 512],
                            in_=kvb_out[128 * h:128 * (h + 1),
                                        j * 512:(j + 1) * 512])
                for kt in range(NKT):
                    nc.sync.dma_start(
                        out=v9[:, kt * 288:(kt + 1) * 288],
                        in_=kvb_out[128 * (kt // 4):128 * (kt // 4) + 128,
                                    4096 + (kt % 4) * 288:
                                    4096 + (kt % 4 + 1) * 288])

            def scores_exp(j):
                e_tiles = []
                for kt in range(NKT):
                    ps_s = sp.tile([128, 2048], F32, name=f"s{j}{kt}", tag="s")
                    for g in range(4):
                        mm(out=ps_s[:, g * 512:(g + 1) * 512],
                           lhsT=kT[32 * g:32 * g + 8,
                                   j * N + kt * 128:j * N + (kt + 1) * 128],
                           rhs=qT[32 * g:32 * g + 8, j * NQ:(j + 1) * NQ],
                           start=True, stop=True,
                           tile_position=(32 * g, 0))
                    e = ep.tile([128, 2048], BF16, name=f"e{j}{kt}", tag="e")
                    nc.scalar.activation(out=e, in_=ps_s, func=EXP, scale=SCALE)
                    e_tiles.append(e)
                return e_tiles

            def attnv(j, e_tiles):
                ps_o = sp.tile([128, 512], F32, name=f"o{j}", tag="s")
                for kt in range(NKT):
                    for g in range(4):
                        mm(out=ps_o[32 * g:32 * g + 9, :],
                           lhsT=v9[:, kt * 288 + (4 * j + g) * 9:
                                   kt * 288 + (4 * j + g) * 9 + 9],
                           rhs=e_tiles[kt][:, g * 512:(g + 1) * 512],
                           start=(kt == 0), stop=(kt == NKT - 1),
                           tile_position=(0, 32 * g))
                o_st = ep.tile([128, 512], F32, name=f"ost{j}", tag="ost")
                nc.vector.tensor_copy(o_st, ps_o)
                # channel of head 4j+g, dim d = 32j+8g+d; cat chunk = j//4
                cc, jr = j // 4, j % 4
                for g in range(4):
                    nc.sync.dma_start(
                        out=attn_cat[32 * jr + 8 * g:32 * jr + 8 * g + 8,
                                     cc * 512:(cc + 1) * 512],
                        in_=o_st[32 * g:32 * g + 8, :])
                    nc.sync.dma_start(
                        out=s_cat[4 * j + g:4 * j + g + 1, :],
                        in_=o_st[32 * g + 8:32 * g + 9, :])

            # k/v projections feed the pair-gather (critical path); q
            # projections overlap with the collective.
            for j in range(NJ):
                proj_k(j)
            proj_v()
            gather_kv()
            for j in range(NJ):
                proj_q(j)
            for j in range(NJ):
                attnv(j, scores_exp(j))

            # ---- tail: normalize + projection ----
            nc.vector.reciprocal(r_cat, s_cat)
            for cc in range(2):
                nc.gpsimd.dma_start(
                    out=rb[:, cc * 512:(cc + 1) * 512],
                    in_=r_cat[16 * cc:16 * cc + 16].unsqueeze(1)
                        .broadcast_to([16, 8, NQ]))
            nc.vector.tensor_mul(attn_n, attn_cat, rb)
            for ot in range(2):
                ps_p = sp.tile([128, 512], F32, name=f"pp{ot}", tag="s")
                for cc in range(2):
                    mm(out=ps_p,
                       lhsT=wpd[:, cc * 256 + ot * 128:
                                cc * 256 + (ot + 1) * 128],
                       rhs=attn_n[:, cc * 512:(cc + 1) * 512],
                       start=(cc == 0), stop=(cc == 1))
                nc.vector.tensor_copy(
                    out_sb[:, ot * 512:(ot + 1) * 512], ps_p)
            for ot in range(2):
                nc.sync.dma_start(
                    out=out_d.ap()[ot * 128:(ot + 1) * 128, :],
                    in_=out_sb[:, ot * 512:(ot + 1) * 512])

    nc.compile()
    _cache["nc"] = nc
    return nc


def _get_compiled():
    if "compiled" in _cache:
        return _cache["compiled"]
    import jax
    import jax.numpy as jnp
    from jax.sharding import Mesh, PartitionSpec, NamedSharding
    from jax.experimental.shard_map import shard_map
    from concourse import bass2jax, mybir

    nc = _build()
    bass2jax.install_neuronx_cc_hook()
    partition_name = (nc.partition_id_tensor.name
                      if nc.partition_id_tensor else None)
    in_names, out_names, out_avals, zero_shapes = [], [], [], []
    for alloc in nc.m.functions[0].allocations:
        if not isinstance(alloc, mybir.MemoryLocationSet):
            continue
        name = alloc.memorylocations[0].name
        if alloc.kind == "ExternalInput":
            if name != partition_name:
                in_names.append(name)
        elif alloc.kind == "ExternalOutput":
            shape = tuple(alloc.tensor_shape)
            dtype = mybir.dt.np(alloc.dtype)
            out_names.append(name)
            out_avals.append(jax.core.ShapedArray(shape, dtype))
            zero_shapes.append((shape, dtype))
    n_params = len(in_names)
    all_in = in_names + out_names + ([partition_name] if partition_name else [])

    def _body(*args):
        operands = list(args)
        if partition_name is not None:
            operands.append(bass2jax.partition_id_tensor())
        outs = bass2jax._bass_exec_p.bind(
            *operands, out_avals=tuple(out_avals), in_names=tuple(all_in),
            out_names=tuple(out_names), lowering_input_output_aliases=(),
            sim_require_finite=True, sim_require_nnan=True, nc=nc)
        return tuple(outs)

    devices = jax.devices()[:NCORES]
    mesh = Mesh(np.asarray(devices), ("core",))
    n_outs = len(out_avals)
    # No donation: the kernel writes every output element, so the zero
    # "output operand" is never read -- keep one device-resident dummy and
    # reuse it every call instead of shipping/creating zeros per call.
    sharded = jax.jit(
        shard_map(_body, mesh=mesh,
                  in_specs=(PartitionSpec("core"),) * (n_params + n_outs),
                  out_specs=(PartitionSpec("core"),) * n_outs,
                  check_rep=False),
        keep_unused=True)

    import ml_dtypes
    in_shapes = {"xq": (C, NQ), "x": (C, NQ), "w": (32, 1024)}
    dummy_in = [np.zeros((NCORES * in_shapes[nm][0], in_shapes[nm][1]),
                         ml_dtypes.bfloat16) for nm in in_names]
    dummy_zero = [np.zeros((NCORES * s[0], *s[1:]), d)
                  for (s, d) in zero_shapes]
    compiled = sharded.lower(*dummy_in, *dummy_zero).compile()

    sh = NamedSharding(mesh, PartitionSpec("core"))
    zeros = [jax.jit(
        (lambda s_, d_: (lambda: jnp.zeros((NCORES * s_[0], *s_[1:]), d_)))(
            s, d), out_shardings=sh)() for (s, d) in zero_shapes]
    for z in zeros:
        z.block_until_ready()

    _cache["compiled"] = (compiled, in_names, out_names, zeros)
    return _cache["compiled"]


def _prep(inputs):
    import ml_dtypes
    bf16 = ml_dtypes.bfloat16
    xq = np.asarray(inputs["xq"]).reshape(B, C, N).astype(bf16)
    x = np.asarray(inputs["x"]).reshape(B, C, N).astype(bf16)
    Wq = np.asarray(inputs["Wq"]).T
    Wkv = np.asarray(inputs["Wkv"])
    Wp = np.asarray(inputs["Wproj"]).T

    # per-core concat along axis 0: core = 2*b + t owns q/kv tokens
    # [512t, 512(t+1)) of batch b -- fully disjoint shards.
    xq_cc = xq.reshape(B, C, 2, NQ).transpose(0, 2, 1, 3).reshape(
        NCORES * C, NQ)
    x_cc = x.reshape(B, C, 2, NQ).transpose(0, 2, 1, 3).reshape(
        NCORES * C, NQ)
    # wall [256, 1024] = [WqT | WkT | WvT | WpT]; concat of 8 row-slices
    # (32 rows/core) along axis 0 is the wall itself.
    wall = np.concatenate([Wq, Wkv[:C].T, Wkv[C:].T, Wp],
                          axis=1).astype(bf16)
    return {"xq": np.ascontiguousarray(xq_cc), "x": np.ascontiguousarray(x_cc),
            "w": wall}


def run_internal(inputs, trace=False):
    compiled, in_names, out_names, zeros = _get_compiled()
    arrs = _prep(inputs)
    concat_in = [arrs[nm] for nm in in_names]
    outs = compiled(*concat_in, *zeros)
    res = np.asarray(outs[0]).astype(np.float32).reshape(NCORES, C, NQ)
    bproj = np.asarray(inputs["bproj"], np.float32)
    out = np.empty((B, C, 32, 32), np.float32)
    for b in range(B):
        full = np.concatenate([res[2 * b], res[2 * b + 1]], axis=1)
        out[b] = (full + bproj[:, None]).reshape(C, 32, 32)
    return out, None


def kernel(**inputs):
    out, _ = run_internal(inputs)
    return out
